# revision 14
# baseline (speedup 1.0000x reference)
"""Causal self-attention (RoPE, 16 heads) on 8 TRN2 NeuronCores.

Sharding: core c handles batch b = c//2 and head half (c%2)*8..+8.
Each core computes a partial output projection (T, C); the host sums
the two partials per batch. No on-device collectives.

All matmuls run as float32r (full PE rate at N>=256, ~1.5e-4 rel err).
Softmax runs unnormalized with a ones-column appended to V (M=65 AV
matmuls produce y and the row-sum together); normalization is applied
after attention via reciprocal + partition broadcast.
"""

import sys
from contextlib import ExitStack

import numpy as np

sys.path.insert(0, "/opt/trn_rl_repo")

import concourse.bacc as bacc
import concourse.mybir as mybir
import concourse.tile as tile
from concourse.bass_utils import run_bass_kernel_spmd

F32 = mybir.dt.float32
F32R = mybir.dt.float32r
EXP = mybir.ActivationFunctionType.Exp

# Problem constants
B, T, C = 4, 2048, 1024
H = 16          # global heads
D = 64          # head dim
HL = 8          # heads per core
N_CORES = 8
ROPE_BASE = 10000.0
SCALE = 1.0 / 8.0  # 1/sqrt(D)

# Derived tiling
TCH = 512            # T-chunk (q-chunk) width
NT = T // TCH        # 4 T-chunks
NJT = T // 128       # 16 k-tiles
CB = C // 128        # 8 contraction chunks
VW = D + 1           # V columns per head incl. ones column


def _build_program(tt=T):
    """Build the SPMD program for sequence length tt (tt % 512 == 0)."""
    nt = tt // TCH
    njt = tt // 128

    nc = bacc.Bacc("TRN2", target_bir_lowering=False, debug=False)
    xt = nc.dram_tensor("xt", (C, tt), F32R, kind="ExternalInput").ap()
    wqkv = nc.dram_tensor("wqkv", (C, 3 * C // 2), F32R, kind="ExternalInput").ap()
    wproj = nc.dram_tensor("wproj", (C // 2, C), F32R, kind="ExternalInput").ap()
    cosr = nc.dram_tensor("cosr", (128, tt), F32, kind="ExternalInput").ap()
    sinr = nc.dram_tensor("sinr", (128, tt), F32, kind="ExternalInput").ap()
    trimask = nc.dram_tensor("trimask", (128, 128), F32, kind="ExternalInput").ap()
    vones = nc.dram_tensor("vones", (128, (tt // 128) * HL), F32R, kind="ExternalInput").ap()
    out = nc.dram_tensor("out", (tt, C), F32, kind="ExternalOutput").ap()

    xt_r = xt.rearrange("(cb p) t -> p cb t", p=128)       # [128, 8, tt]
    wqkv_r = wqkv.rearrange("(cb p) f -> p cb f", p=128)   # [128, 8, 1536]
    wproj_r = wproj.rearrange("(m p) o -> p m o", p=128)   # [128, 4, 1024]

    with tile.TileContext(nc) as tc, ExitStack() as ctx:
        # ---- persistent buffers ----
        persist = ctx.enter_context(tc.tile_pool(name="persist", bufs=1))
        qtr = [persist.tile([128, tt], F32R, name=f"qtr{i}", tag=f"qtr{i}")
               for i in range(4)]
        ktr = [persist.tile([128, tt], F32R, name=f"ktr{i}", tag=f"ktr{i}")
               for i in range(4)]
        vp = persist.tile([128, njt * VW * HL], F32R, name="vp", tag="vp")
        tri = persist.tile([128, 128], F32, name="tri", tag="tri")

        nc.sync.dma_start(out=tri, in_=trimask)

        # ones columns of Vp: col j*VW*HL + lh*VW + D for all j, lh
        vp_r = vp.rearrange("p (j lh w) -> p j lh w", j=njt, lh=HL)
        nc.sync.dma_start(out=vp_r[:, :, :, D:D + 1],
                          in_=vones.rearrange("p (j lh) -> p j lh", j=njt))

        # ================= Phase 1: QKV + RoPE + repack =================
        with ExitStack() as p1:
            xt_pool = p1.enter_context(tc.tile_pool(name="xt", bufs=2))
            w_pool = p1.enter_context(tc.tile_pool(name="w", bufs=2))
            rp_pool = p1.enter_context(tc.tile_pool(name="rope", bufs=2))
            cssn_pool = p1.enter_context(tc.tile_pool(name="cssn", bufs=1))
            ps_qk = p1.enter_context(tc.tile_pool(name="psqk", bufs=2, space="PSUM"))
            ps_v = p1.enter_context(tc.tile_pool(name="psv", bufs=2, space="PSUM"))

            cs_t = cssn_pool.tile([128, tt], F32, name="cs", tag="cs")
            sn_t = cssn_pool.tile([128, tt], F32, name="sn", tag="sn")
            nc.sync.dma_start(out=cs_t, in_=cosr)
            nc.sync.dma_start(out=sn_t, in_=sinr)

            for tci in range(nt):
                tsl = slice(tci * TCH, (tci + 1) * TCH)
                x_t = xt_pool.tile([128, CB, TCH], F32R)
                for cb in range(CB):
                    nc.sync.dma_start(out=x_t[:, cb, :], in_=xt_r[:, cb, tsl])

                # Q and K feature groups: fg 0,1 -> Q heads 0-3, 4-7 (even|odd)
                #                          fg 2,3 -> K heads 0-3, 4-7
                for fg in range(4):
                    w_t = w_pool.tile([128, CB, 256], F32R)
                    for cb in range(CB):
                        nc.sync.dma_start(
                            out=w_t[:, cb, :],
                            in_=wqkv_r[:, cb, fg * 256:(fg + 1) * 256])
                    ps_e = ps_qk.tile([128, TCH], F32, name="pse", tag="pse")
                    ps_o = ps_qk.tile([128, TCH], F32, name="pso", tag="pso")
                    for cb in range(CB):
                        nc.tensor.matmul(
                            ps_e, w_t[:, cb, 0:128],
                            x_t[:, cb, :],
                            start=(cb == 0), stop=(cb == CB - 1))
                    for cb in range(CB):
                        nc.tensor.matmul(
                            ps_o, w_t[:, cb, 128:256],
                            x_t[:, cb, :],
                            start=(cb == 0), stop=(cb == CB - 1))
                    # RoPE: yE = E*cos - O*sin ; yO = E*sin + O*cos
                    cs = cs_t[:, tsl]
                    sn = sn_t[:, tsl]
                    t1 = rp_pool.tile([128, TCH], F32R, name="t1")
                    t2 = rp_pool.tile([128, TCH], F32, name="t2")
                    t3 = rp_pool.tile([128, TCH], F32R, name="t3")
                    t4 = rp_pool.tile([128, TCH], F32, name="t4")
                    nc.vector.tensor_mul(t1, ps_e, cs)
                    nc.vector.tensor_mul(t2, ps_o, sn)
                    nc.vector.tensor_sub(t1, t1, t2)   # yE
                    nc.vector.tensor_mul(t3, ps_e, sn)
                    nc.vector.tensor_mul(t4, ps_o, cs)
                    nc.vector.tensor_add(t3, t3, t4)   # yO
                    # repack: head lh4 (0..3 within group) -> chunk, partition
                    dst = qtr if fg < 2 else ktr
                    hg = fg % 2  # head group (heads 4*hg .. 4*hg+3)
                    for lh4 in range(4):
                        lh = 4 * hg + lh4
                        ch = dst[lh // 2]
                        pb = 64 * (lh % 2)
                        nc.sync.dma_start(
                            out=ch[pb:pb + 32, tsl],
                            in_=t1[32 * lh4:32 * lh4 + 32, :])
                        nc.sync.dma_start(
                            out=ch[pb + 32:pb + 64, tsl],
                            in_=t3[32 * lh4:32 * lh4 + 32, :])

                # V in natural [t, f] layout, 2 halves of 256 features
                for vh in range(2):
                    w_t = w_pool.tile([128, CB, 256], F32R)
                    for cb in range(CB):
                        nc.sync.dma_start(
                            out=w_t[:, cb, :],
                            in_=wqkv_r[:, cb, C + vh * 256:C + (vh + 1) * 256])
                    for tt4 in range(4):
                        jt = tci * 4 + tt4
                        ps_vv = ps_v.tile([128, 256], F32, name="psvv", tag="psvv")
                        for cb in range(CB):
                            nc.tensor.matmul(
                                ps_vv,
                                x_t[:, cb, tt4 * 128:(tt4 + 1) * 128],
                                w_t[:, cb, :],
                                start=(cb == 0), stop=(cb == CB - 1))
                        # copy into Vp: 4 heads (vh*4..vh*4+3), 64 cols each,
                        # strided by VW to leave the ones column
                        dst = vp_r[:, jt, vh * 4:vh * 4 + 4, 0:D]
                        nc.vector.tensor_copy(dst, ps_vv.rearrange(
                            "p (h d) -> p h d", h=4))

        # ================= Phase 2: attention + proj =================
        with ExitStack() as p2:
            y_pool = p2.enter_context(tc.tile_pool(name="ytnp", bufs=1))
            e_pool = p2.enter_context(tc.tile_pool(name="expt", bufs=2))
            n_pool = p2.enter_context(tc.tile_pool(name="norm", bufs=2))
            o_pool = p2.enter_context(tc.tile_pool(name="outc", bufs=4))
            ps_s = p2.enter_context(tc.tile_pool(name="pss", bufs=2, space="PSUM"))
            ps_y = p2.enter_context(tc.tile_pool(name="psy", bufs=2, space="PSUM"))

            ytn = [y_pool.tile([128, tt], F32R, name=f"ytn{i}", tag=f"ytn{i}")
                   for i in range(4)]
            wp_t = y_pool.tile([128, 4, C], F32R, name="wp", tag="wp")
            for m in range(4):
                nc.sync.dma_start(out=wp_t[:, m, :], in_=wproj_r[:, m, :])

            for qc in range(nt):
                qbase = qc * TCH
                for pr in range(4):
                    h_a, h_b = 2 * pr, 2 * pr + 1
                    y_a = ps_y.tile([D + 1, TCH], F32, name="ya", tag="ya")
                    y_b = ps_y.tile([D + 1, TCH], F32, name="yb", tag="yb")
                    njs = 4 * qc + 4
                    for j in range(njs):
                        r = j - 4 * qc
                        q0 = 128 * max(r, 0)
                        qsl = slice(qbase + q0, qbase + TCH)
                        jsl = slice(j * 128, (j + 1) * 128)
                        s_ab = ps_s.tile([128, 2 * TCH], F32, name="sab", tag="sab")
                        nc.tensor.matmul(
                            s_ab[:, q0:TCH],
                            ktr[pr][0:64, jsl],
                            qtr[pr][0:64, qsl],
                            start=True, stop=True)
                        nc.tensor.matmul(
                            s_ab[:, TCH + q0:2 * TCH],
                            ktr[pr][64:128, jsl],
                            qtr[pr][64:128, qsl],
                            start=True, stop=True)
                        e_ab = e_pool.tile([128, 2 * TCH], F32R, name="eab")
                        if q0 == 0:
                            nc.scalar.activation(e_ab, s_ab, EXP, scale=SCALE)
                        else:
                            nc.scalar.activation(e_ab[:, q0:TCH],
                                                 s_ab[:, q0:TCH], EXP,
                                                 scale=SCALE)
                            nc.scalar.activation(e_ab[:, TCH + q0:2 * TCH],
                                                 s_ab[:, TCH + q0:2 * TCH], EXP,
                                                 scale=SCALE)
                        if r >= 0:
                            nc.vector.tensor_mul(
                                e_ab[:, q0:q0 + 128], e_ab[:, q0:q0 + 128], tri)
                            nc.vector.tensor_mul(
                                e_ab[:, TCH + q0:TCH + q0 + 128],
                                e_ab[:, TCH + q0:TCH + q0 + 128], tri)
                        vb = j * VW * HL
                        nc.tensor.matmul(
                            y_a[:, q0:TCH],
                            vp[:, vb + h_a * VW:vb + (h_a + 1) * VW],
                            e_ab[:, q0:TCH],
                            start=(j == 0), stop=(j == njs - 1))
                        nc.tensor.matmul(
                            y_b[:, q0:TCH],
                            vp[:, vb + h_b * VW:vb + (h_b + 1) * VW],
                            e_ab[:, TCH + q0:2 * TCH],
                            start=(j == 0), stop=(j == njs - 1))
                    for h, y_t in ((h_a, y_a), (h_b, y_b)):
                        rt = n_pool.tile([D + 1, TCH], F32, name="rt")
                        nc.vector.reciprocal(rt[D:D + 1, :], y_t[D:D + 1, :])
                        r0 = n_pool.tile([1, TCH], F32, name="r0")
                        nc.sync.dma_start(out=r0, in_=rt[D:D + 1, :])
                        rb = n_pool.tile([D, TCH], F32, name="rb")
                        nc.gpsimd.partition_broadcast(rb, r0)
                        yn = n_pool.tile([D, TCH], F32R, name="yn")
                        nc.vector.tensor_mul(yn, y_t[0:D, :], rb)
                        nc.sync.dma_start(
                            out=ytn[h // 2][64 * (h % 2):64 * (h % 2) + 64,
                                            qbase:qbase + TCH],
                            in_=yn)
                # output projection for this q-chunk
                for tt4 in range(4):
                    tsl = slice(qbase + tt4 * 128, qbase + (tt4 + 1) * 128)
                    for oc in range(2):
                        pp = ps_s.tile([128, TCH], F32, name="pp", tag="sab")
                        for m in range(4):
                            nc.tensor.matmul(
                                pp, ytn[m][:, tsl],
                                wp_t[:, m, oc * TCH:(oc + 1) * TCH],
                                start=(m == 0), stop=(m == 3))
                        ot = o_pool.tile([128, TCH], F32, name="ot")
                        nc.vector.tensor_copy(ot, pp)
                        nc.sync.dma_start(
                            out=out[tsl, oc * TCH:(oc + 1) * TCH], in_=ot)

    nc.compile()
    return nc


def _prep_inputs(x, w_attn, w_proj, tt=T):
    """Per-core host sharding. Returns in_maps list."""
    x = np.asarray(x, dtype=np.float32)
    w_attn = np.asarray(w_attn, dtype=np.float32)
    w_proj = np.asarray(w_proj, dtype=np.float32)

    # rope tables [128, tt]: row p -> pair index p % 32
    ip = np.arange(128) % 32
    inv = ROPE_BASE ** (-(2.0 * ip) / D)
    t_idx = np.arange(tt, dtype=np.float64)
    ang = t_idx[None, :] * inv[:, None]
    cosr = np.cos(ang).astype(np.float32)
    sinr = np.sin(ang).astype(np.float32)

    k_idx = np.arange(128)
    q_idx = np.arange(128)
    trimask = (k_idx[:, None] <= q_idx[None, :]).astype(np.float32)

    in_maps = []
    for c in range(N_CORES):
        b = c // 2
        hb = (c % 2) * HL
        # Q/K column permutation: fg groups (Q g0 even|odd, Q g1, K g0, K g1)
        cols = []
        for qk in range(2):  # 0=Q, 1=K
            for g in range(2):
                for par in range(2):  # 0=even dims, 1=odd dims
                    for lh4 in range(4):
                        h = hb + 4 * g + lh4
                        for i in range(32):
                            cols.append(qk * C + h * D + 2 * i + par)
        for lh in range(HL):  # V natural
            h = hb + lh
            for d in range(D):
                cols.append(2 * C + h * D + d)
        wqkv_c = np.ascontiguousarray(w_attn[:, cols])
        wproj_c = np.ascontiguousarray(w_proj[hb * D:(hb + HL) * D, :])
        xt_c = np.ascontiguousarray(x[b, :tt].T)
        in_maps.append({
            "xt": xt_c, "wqkv": wqkv_c, "wproj": wproj_c,
            "cosr": cosr, "sinr": sinr, "trimask": trimask,
            "vones": np.ones((128, (tt // 128) * HL), dtype=np.float32),
        })
    return in_maps


_PROGRAM_CACHE = {}


def _get_program(tt=T):
    if tt not in _PROGRAM_CACHE:
        _PROGRAM_CACHE[tt] = _build_program(tt)
    return _PROGRAM_CACHE[tt]


def run(x, w_attn, w_proj, tt=T, **run_kwargs):
    nc = _get_program(tt)
    in_maps = _prep_inputs(x, w_attn, w_proj, tt)
    res = run_bass_kernel_spmd(nc, in_maps, core_ids=list(range(N_CORES)),
                               **run_kwargs)
    parts = [res.results[c]["out"] for c in range(N_CORES)]
    y = np.stack([parts[2 * b] + parts[2 * b + 1] for b in range(B)])
    return y, res


def kernel(x, W_attn, W_proj):
    y, _ = run(x, W_attn, W_proj, tt=T)
    return y


# revision 19
# speedup vs baseline: 1.1024x; 1.1024x over previous
"""Causal self-attention (RoPE, 16 heads) on 8 TRN2 NeuronCores.

Sharding: core c handles batch b = c//2 and head half (c%2)*8..+8.
Each core computes a partial output projection (T, C); the host sums
the two partials per batch. No on-device collectives.

All matmuls run as float32r (full PE rate at N>=256, ~1.5e-4 rel err).
Softmax runs unnormalized with a ones-column appended to V (M=65 AV
matmuls produce y and the row-sum together); normalization is applied
after attention via reciprocal + partition broadcast.
"""

import sys
from contextlib import ExitStack

import numpy as np

sys.path.insert(0, "/opt/trn_rl_repo")

import concourse.bacc as bacc
import concourse.mybir as mybir
import concourse.tile as tile
from concourse.bass_utils import run_bass_kernel_spmd

F32 = mybir.dt.float32
F32R = mybir.dt.float32r
EXP = mybir.ActivationFunctionType.Exp

# Problem constants
B, T, C = 4, 2048, 1024
H = 16          # global heads
D = 64          # head dim
HL = 8          # heads per core
N_CORES = 8
ROPE_BASE = 10000.0
SCALE = 1.0 / 8.0  # 1/sqrt(D)

# Derived tiling
TCH = 512            # T-chunk (q-chunk) width
NT = T // TCH        # 4 T-chunks
NJT = T // 128       # 16 k-tiles
CB = C // 128        # 8 contraction chunks
VW = D + 1           # V columns per head incl. ones column


def _build_program(tt=T):
    """Build the SPMD program for sequence length tt (tt % 512 == 0)."""
    nt = tt // TCH
    njt = tt // 128

    nc = bacc.Bacc("TRN2", target_bir_lowering=False, debug=False)
    xt = nc.dram_tensor("xt", (C, tt), F32R, kind="ExternalInput").ap()
    wqkv = nc.dram_tensor("wqkv", (C, 3 * C // 2), F32R, kind="ExternalInput").ap()
    wproj = nc.dram_tensor("wproj", (C // 2, C), F32R, kind="ExternalInput").ap()
    cosr = nc.dram_tensor("cosr", (128, tt), F32, kind="ExternalInput").ap()
    sinr = nc.dram_tensor("sinr", (128, tt), F32, kind="ExternalInput").ap()
    trimask = nc.dram_tensor("trimask", (128, 128), F32, kind="ExternalInput").ap()
    vones = nc.dram_tensor("vones", (128, (tt // 128) * HL), F32R, kind="ExternalInput").ap()
    out = nc.dram_tensor("out", (tt, C), F32, kind="ExternalOutput").ap()

    xt_r = xt.rearrange("(cb p) t -> p cb t", p=128)       # [128, 8, tt]
    wqkv_r = wqkv.rearrange("(cb p) f -> p cb f", p=128)   # [128, 8, 1536]
    wproj_r = wproj.rearrange("(m p) o -> p m o", p=128)   # [128, 4, 1024]

    with tile.TileContext(nc) as tc, ExitStack() as ctx:
        # ---- persistent buffers ----
        persist = ctx.enter_context(tc.tile_pool(name="persist", bufs=1))
        qtr = [persist.tile([128, tt], F32R, name=f"qtr{i}", tag=f"qtr{i}")
               for i in range(4)]
        ktr = [persist.tile([128, tt], F32R, name=f"ktr{i}", tag=f"ktr{i}")
               for i in range(4)]
        vp = persist.tile([128, njt * VW * HL], F32R, name="vp", tag="vp")
        tri = persist.tile([128, 128], F32, name="tri", tag="tri")

        nc.sync.dma_start(out=tri, in_=trimask)

        # ones columns of Vp: col j*VW*HL + lh*VW + D for all j, lh
        vp_r = vp.rearrange("p (j lh w) -> p j lh w", j=njt, lh=HL)
        nc.sync.dma_start(out=vp_r[:, :, :, D:D + 1],
                          in_=vones.rearrange("p (j lh) -> p j lh", j=njt))

        # ================= Phase 1: QKV + RoPE + repack =================
        with ExitStack() as p1:
            xt_pool = p1.enter_context(tc.tile_pool(name="xt", bufs=2))
            w_pool = p1.enter_context(tc.tile_pool(name="w", bufs=2))
            rp_pool = p1.enter_context(tc.tile_pool(name="rope", bufs=2))
            cssn_pool = p1.enter_context(tc.tile_pool(name="cssn", bufs=1))
            ps_qk = p1.enter_context(tc.tile_pool(name="psqk", bufs=2, space="PSUM"))
            ps_v = p1.enter_context(tc.tile_pool(name="psv", bufs=2, space="PSUM"))

            cs_t = cssn_pool.tile([128, tt], F32, name="cs", tag="cs")
            sn_t = cssn_pool.tile([128, tt], F32, name="sn", tag="sn")
            nc.sync.dma_start(out=cs_t, in_=cosr)
            nc.sync.dma_start(out=sn_t, in_=sinr)

            def qk_group(fg, tci, x_t, w_t):
                tsl = slice(tci * TCH, (tci + 1) * TCH)
                ps_e = ps_qk.tile([128, TCH], F32, name="pse", tag="pse")
                ps_o = ps_qk.tile([128, TCH], F32, name="pso", tag="pso")
                for cb in range(CB):
                    nc.tensor.matmul(
                        ps_e, w_t[:, cb, 0:128], x_t[:, cb, :],
                        start=(cb == 0), stop=(cb == CB - 1))
                for cb in range(CB):
                    nc.tensor.matmul(
                        ps_o, w_t[:, cb, 128:256], x_t[:, cb, :],
                        start=(cb == 0), stop=(cb == CB - 1))
                # RoPE: yE = E*cos - O*sin ; yO = E*sin + O*cos
                cs = cs_t[:, tsl]
                sn = sn_t[:, tsl]
                t1 = rp_pool.tile([128, TCH], F32R, name="t1")
                t2 = rp_pool.tile([128, TCH], F32, name="t2")
                t3 = rp_pool.tile([128, TCH], F32R, name="t3")
                t4 = rp_pool.tile([128, TCH], F32, name="t4")
                nc.vector.tensor_mul(t1, ps_e, cs)
                nc.vector.tensor_mul(t2, ps_o, sn)
                nc.vector.tensor_sub(t1, t1, t2)   # yE
                nc.vector.tensor_mul(t3, ps_e, sn)
                nc.vector.tensor_mul(t4, ps_o, cs)
                nc.vector.tensor_add(t3, t3, t4)   # yO
                # repack: head lh4 (0..3 within group) -> chunk, partition
                dst = qtr if fg < 2 else ktr
                hg = fg % 2
                for lh4 in range(4):
                    lh = 4 * hg + lh4
                    ch = dst[lh // 2]
                    pb = 64 * (lh % 2)
                    nc.sync.dma_start(
                        out=ch[pb:pb + 32, tsl],
                        in_=t1[32 * lh4:32 * lh4 + 32, :])
                    nc.sync.dma_start(
                        out=ch[pb + 32:pb + 64, tsl],
                        in_=t3[32 * lh4:32 * lh4 + 32, :])

            def v_block(vh, tci, x_t, w_t):
                for tt4 in range(4):
                    jt = tci * 4 + tt4
                    ps_vv = ps_v.tile([128, 256], F32, name="psvv", tag="psvv")
                    for cb in range(CB):
                        nc.tensor.matmul(
                            ps_vv,
                            x_t[:, cb, tt4 * 128:(tt4 + 1) * 128],
                            w_t[:, cb, :],
                            start=(cb == 0), stop=(cb == CB - 1))
                    # copy into Vp: 4 heads (vh*4..vh*4+3), 64 cols each,
                    # strided by VW to leave the ones column
                    dstv = vp_r[:, jt, vh * 4:vh * 4 + 4, 0:D]
                    nc.vector.tensor_copy(dstv, ps_vv.rearrange(
                        "p (h d) -> p h d", h=4))

            # T-pairs share one load of W (halves W traffic)
            for g0 in range(0, nt, 2):
                grp = list(range(g0, min(g0 + 2, nt)))
                xts = {}
                for tci in grp:
                    x_t = xt_pool.tile([128, CB, TCH], F32R, name="xtile")
                    for cb in range(CB):
                        nc.sync.dma_start(
                            out=x_t[:, cb, :],
                            in_=xt_r[:, cb, tci * TCH:(tci + 1) * TCH])
                    xts[tci] = x_t
                # Q and K feature groups: fg 0,1 -> Q heads 0-3, 4-7
                # (even|odd); fg 2,3 -> K heads 0-3, 4-7
                for fg in range(4):
                    w_t = w_pool.tile([128, CB, 256], F32R, name="wtile")
                    for cb in range(CB):
                        nc.sync.dma_start(
                            out=w_t[:, cb, :],
                            in_=wqkv_r[:, cb, fg * 256:(fg + 1) * 256])
                    for tci in grp:
                        qk_group(fg, tci, xts[tci], w_t)
                # V in natural [t, f] layout, 2 halves of 256 features
                for vh in range(2):
                    w_t = w_pool.tile([128, CB, 256], F32R, name="wtile")
                    for cb in range(CB):
                        nc.sync.dma_start(
                            out=w_t[:, cb, :],
                            in_=wqkv_r[:, cb, C + vh * 256:C + (vh + 1) * 256])
                    for tci in grp:
                        v_block(vh, tci, xts[tci], w_t)

        # ================= Phase 2: attention + proj =================
        with ExitStack() as p2:
            y_pool = p2.enter_context(tc.tile_pool(name="ytnp", bufs=1))
            e_pool = p2.enter_context(tc.tile_pool(name="expt", bufs=2))
            n_pool = p2.enter_context(tc.tile_pool(name="norm", bufs=2))
            o_pool = p2.enter_context(tc.tile_pool(name="outc", bufs=4))
            ps_s = p2.enter_context(tc.tile_pool(name="pss", bufs=2, space="PSUM"))
            ps_y = p2.enter_context(tc.tile_pool(name="psy", bufs=3, space="PSUM"))
            ps_p = p2.enter_context(tc.tile_pool(name="psp", bufs=1, space="PSUM"))

            ytn = [y_pool.tile([128, tt], F32R, name=f"ytn{i}", tag=f"ytn{i}")
                   for i in range(4)]
            wp_t = y_pool.tile([128, 4, C], F32R, name="wp", tag="wp")
            for m in range(4):
                nc.sync.dma_start(out=wp_t[:, m, :], in_=wproj_r[:, m, :])

            def emit_proj(qc):
                qbase = qc * TCH
                for tt4 in range(4):
                    tsl = slice(qbase + tt4 * 128, qbase + (tt4 + 1) * 128)
                    for oc in range(2):
                        pp = ps_p.tile([128, TCH], F32, name="pp", tag="pp")
                        for m in range(4):
                            nc.tensor.matmul(
                                pp, ytn[m][:, tsl],
                                wp_t[:, m, oc * TCH:(oc + 1) * TCH],
                                start=(m == 0), stop=(m == 3))
                        ot = o_pool.tile([128, TCH], F32, name="ot")
                        nc.vector.tensor_copy(ot, pp)
                        nc.sync.dma_start(
                            out=out[tsl, oc * TCH:(oc + 1) * TCH], in_=ot)

            for qc in range(nt):
                qbase = qc * TCH
                for pr in range(4):
                    h_a, h_b = 2 * pr, 2 * pr + 1
                    y_a = ps_y.tile([D + 1, TCH], F32, name="ya", tag="y")
                    y_b = ps_y.tile([D + 1, TCH], F32, name="yb", tag="y")
                    njs = 4 * qc + 4
                    for j in range(njs):
                        r = j - 4 * qc
                        q0 = 128 * max(r, 0)
                        qsl = slice(qbase + q0, qbase + TCH)
                        jsl = slice(j * 128, (j + 1) * 128)
                        s_ab = ps_s.tile([128, 2 * TCH], F32, name="sab", tag="sab")
                        nc.tensor.matmul(
                            s_ab[:, q0:TCH],
                            ktr[pr][0:64, jsl],
                            qtr[pr][0:64, qsl],
                            start=True, stop=True)
                        nc.tensor.matmul(
                            s_ab[:, TCH + q0:2 * TCH],
                            ktr[pr][64:128, jsl],
                            qtr[pr][64:128, qsl],
                            start=True, stop=True)
                        e_ab = e_pool.tile([128, 2 * TCH], F32R, name="eab")
                        if q0 == 0:
                            nc.scalar.activation(e_ab, s_ab, EXP, scale=SCALE)
                        else:
                            nc.scalar.activation(e_ab[:, q0:TCH],
                                                 s_ab[:, q0:TCH], EXP,
                                                 scale=SCALE)
                            nc.scalar.activation(e_ab[:, TCH + q0:2 * TCH],
                                                 s_ab[:, TCH + q0:2 * TCH], EXP,
                                                 scale=SCALE)
                        if r >= 0:
                            nc.vector.tensor_mul(
                                e_ab[:, q0:q0 + 128], e_ab[:, q0:q0 + 128], tri)
                            nc.vector.tensor_mul(
                                e_ab[:, TCH + q0:TCH + q0 + 128],
                                e_ab[:, TCH + q0:TCH + q0 + 128], tri)
                        vb = j * VW * HL
                        nc.tensor.matmul(
                            y_a[:, q0:TCH],
                            vp[:, vb + h_a * VW:vb + (h_a + 1) * VW],
                            e_ab[:, q0:TCH],
                            start=(j == 0), stop=(j == njs - 1))
                        nc.tensor.matmul(
                            y_b[:, q0:TCH],
                            vp[:, vb + h_b * VW:vb + (h_b + 1) * VW],
                            e_ab[:, TCH + q0:2 * TCH],
                            start=(j == 0), stop=(j == njs - 1))
                    for h, y_t in ((h_a, y_a), (h_b, y_b)):
                        rt = n_pool.tile([D + 1, TCH], F32, name="rt")
                        nc.vector.reciprocal(rt[D:D + 1, :], y_t[D:D + 1, :])
                        r0 = n_pool.tile([1, TCH], F32, name="r0")
                        nc.sync.dma_start(out=r0, in_=rt[D:D + 1, :])
                        rb = n_pool.tile([D, TCH], F32, name="rb")
                        nc.gpsimd.partition_broadcast(rb, r0)
                        yn = n_pool.tile([D, TCH], F32R, name="yn")
                        nc.vector.tensor_mul(yn, y_t[0:D, :], rb)
                        nc.sync.dma_start(
                            out=ytn[h // 2][64 * (h % 2):64 * (h % 2) + 64,
                                            qbase:qbase + TCH],
                            in_=yn)
                    if pr == 0 and qc > 0:
                        # deferred projection of the previous q-chunk: its
                        # inputs are long-ready, so it fills PE gaps here
                        emit_proj(qc - 1)
            emit_proj(nt - 1)

    nc.compile()
    return nc


def _prep_inputs(x, w_attn, w_proj, tt=T):
    """Per-core host sharding. Returns in_maps list."""
    x = np.asarray(x, dtype=np.float32)
    w_attn = np.asarray(w_attn, dtype=np.float32)
    w_proj = np.asarray(w_proj, dtype=np.float32)

    # rope tables [128, tt]: row p -> pair index p % 32
    ip = np.arange(128) % 32
    inv = ROPE_BASE ** (-(2.0 * ip) / D)
    t_idx = np.arange(tt, dtype=np.float64)
    ang = t_idx[None, :] * inv[:, None]
    cosr = np.cos(ang).astype(np.float32)
    sinr = np.sin(ang).astype(np.float32)

    k_idx = np.arange(128)
    q_idx = np.arange(128)
    trimask = (k_idx[:, None] <= q_idx[None, :]).astype(np.float32)

    in_maps = []
    for c in range(N_CORES):
        b = c // 2
        hb = (c % 2) * HL
        # Q/K column permutation: fg groups (Q g0 even|odd, Q g1, K g0, K g1)
        cols = []
        for qk in range(2):  # 0=Q, 1=K
            for g in range(2):
                for par in range(2):  # 0=even dims, 1=odd dims
                    for lh4 in range(4):
                        h = hb + 4 * g + lh4
                        for i in range(32):
                            cols.append(qk * C + h * D + 2 * i + par)
        for lh in range(HL):  # V natural
            h = hb + lh
            for d in range(D):
                cols.append(2 * C + h * D + d)
        wqkv_c = np.ascontiguousarray(w_attn[:, cols])
        wproj_c = np.ascontiguousarray(w_proj[hb * D:(hb + HL) * D, :])
        xt_c = np.ascontiguousarray(x[b, :tt].T)
        in_maps.append({
            "xt": xt_c, "wqkv": wqkv_c, "wproj": wproj_c,
            "cosr": cosr, "sinr": sinr, "trimask": trimask,
            "vones": np.ones((128, (tt // 128) * HL), dtype=np.float32),
        })
    return in_maps


_PROGRAM_CACHE = {}


def _get_program(tt=T):
    if tt not in _PROGRAM_CACHE:
        _PROGRAM_CACHE[tt] = _build_program(tt)
    return _PROGRAM_CACHE[tt]


def run(x, w_attn, w_proj, tt=T, **run_kwargs):
    nc = _get_program(tt)
    in_maps = _prep_inputs(x, w_attn, w_proj, tt)
    res = run_bass_kernel_spmd(nc, in_maps, core_ids=list(range(N_CORES)),
                               **run_kwargs)
    parts = [res.results[c]["out"] for c in range(N_CORES)]
    y = np.stack([parts[2 * b] + parts[2 * b + 1] for b in range(B)])
    return y, res


def kernel(x, W_attn, W_proj):
    y, _ = run(x, W_attn, W_proj, tt=T)
    return y
